# revision 6
# baseline (speedup 1.0000x reference)
"""Trainium2 Bass kernel for a Chemprop GNN message-passing layer.

Reference computation (single layer, n_nodes=50000, n_edges=300000, hidden=256):
    H   = relu(E)                                  # [E, 256]
    M_v = segment_sum(H, dest, n_nodes)            # [V, 256]
    out = (M_v[src] - H[rev]) @ W.T + b            # [E, 256]

Distribution over 8 NeuronCores (zero collectives): nodes are sharded; core c
owns node range [c*6250, (c+1)*6250), as 49 blocks of 128 node lanes.

Host prep (pure permutation / gather / relu, all O(E*H)):
  * Phase 1 (segment sum): edges grouped by dest-node block; relu(E) rows
    written to a [128, NBLK*CPB1*256] f16 slab in (lane, block, chunk, hid)
    order so each block's DMA is contiguous per partition.
  * Phase 2: edges grouped by src-node block. relu(E)[rev] rows are
    PRE-GATHERED on host and stored TRANSPOSED per chunk:
    ERV[p, (blk*CPB2+j)*256 + r*128 + i] = relu(E[rev[slot(blk,j,i)]])[r*128+p].
    This removes all device-side indirect gathers and PE transposes.

Device per block bb (software-pipelined: phase1(bb+1) emitted before
phase2(bb) so PE never stalls on the Mv PSUM->SBUF copy):
  phase 1: stream h_blk; per chunk build one-hot S[e,n]=(dest_lane[e]==n) on
    DVE and accumulate Mv_ps += S.T @ H_chunk on PE; ScalarE copies Mv to a
    resident f16 SBUF tile (49 x [128,256]).
  phase 2: stream erv (already relu'd, transposed, negatable); build
    R[n,e]=(src_lane[e]==n) from a partition_broadcast (GPSIMD) of the src
    lane row; per chunk:
      pv_T[r*128+q, i] = Mv[:, r-half].T @ R  (2 matmuls into one PSUM tile)
      muv_T = pv_T - erv_chunk                (DVE, f16 out)
      out_ps = muv_T[0:128].T @ W.T[0:128] + muv_T[128:256].T @ W.T[128:256]
      ScalarE copies out_ps (f32) -> out_blk (f16)
    One DMA writes the block's outputs; bias is added on host in assemble().
"""

import sys
from contextlib import ExitStack

import numpy as np

sys.path.insert(0, "/opt/trn_rl_repo")

import concourse.bass as bass
import concourse.bacc as bacc
import concourse.tile as tile
from concourse import mybir
from concourse.bass_utils import run_bass_kernel_spmd

N_NODES = 50000
N_EDGES = 300000
HID = 256
NC = 8
P = 128
NPC = N_NODES // NC          # 6250 nodes per core
NBLK = (NPC + P - 1) // P    # 49 blocks of 128 node lanes per core
PAD_LANE = 200.0             # sentinel lane value -> one-hot row of zeros
SB_BUFS = 4
PS_BUFS = (2, 3, 3)          # psum bufs: mv, pv, out


def _groups(cpb):
    """Split cpb chunks into pairs (last group may be a single)."""
    gs = []
    j = 0
    while j < cpb:
        w = min(2, cpb - j)
        gs.append((j, w))
        j += w
    return gs


def _group_slots(node_ids):
    """Group edges by (core, block) of node ownership; assign (chunk, lane)
    slots. Returns (order, core, blk, j, p, lane, CPB)."""
    c = node_ids // NPC
    loc = node_ids - c * NPC
    blk = loc >> 7
    lane = loc & 127
    g = c * NBLK + blk
    order = np.argsort(g, kind="stable")
    gs = g[order]
    starts = np.searchsorted(gs, np.arange(NC * NBLK))
    counts = np.diff(np.append(starts, node_ids.shape[0]))
    CPB = int(-(-counts.max() // P))
    rank = np.arange(node_ids.shape[0]) - starts[gs]
    j = rank >> 7
    p = rank & 127
    return order, c[order], blk[order], j, p, lane[order], int(CPB)


def prepare(E, edge_index, rev_index, W, b):
    """Host-side sharding. Returns (in_maps, meta)."""
    E = np.ascontiguousarray(E, dtype=np.float32)
    src = np.asarray(edge_index[0], dtype=np.int64)
    dest = np.asarray(edge_index[1], dtype=np.int64)
    rev = np.asarray(rev_index, dtype=np.int64)
    W = np.asarray(W, dtype=np.float32)
    b = np.asarray(b, dtype=np.float32)

    reluE = np.maximum(E, 0.0).astype(np.float16)

    o1, c1, blk1, j1, p1, lane1, CPB1 = _group_slots(dest)
    col1 = blk1 * CPB1 + j1
    o2, c2, blk2, j2, p2, lane2, CPB2 = _group_slots(src)
    col2 = blk2 * CPB2 + j2

    Wt_stack = np.ascontiguousarray(W.T.reshape(2, P, HID)).astype(np.float16)
    iota_row = np.ascontiguousarray(
        np.broadcast_to(np.arange(P, dtype=np.float32), (P, P))).astype(
        np.float16)
    iota_col = np.arange(P, dtype=np.float16).reshape(P, 1)

    in_maps = []
    metas = []
    for c in range(NC):
        m1 = c1 == c
        e1 = o1[m1]
        E_p1 = np.zeros((P, NBLK * CPB1, HID), np.float16)
        E_p1[p1[m1], col1[m1]] = reluE[e1]
        dest_lanes = np.full((P, NBLK * CPB1), PAD_LANE, np.float16)
        dest_lanes[p1[m1], col1[m1]] = lane1[m1].astype(np.float16)

        m2 = c2 == c
        e2 = o2[m2]
        nm = e2.shape[0]
        # ERV layout matches the paired pv PSUM tiles: for chunk j in group
        # (og, w), jj = j - og:
        #   ERV[q, blk*CPB2*256 + og*256 + r*w*128 + jj*128 + i]
        #     = relu(E[rev[slot(blk, j, i)]])[r*128 + q]
        gps = _groups(CPB2)
        og_of = np.empty(CPB2, np.int64)
        w_of = np.empty(CPB2, np.int64)
        for og, w in gps:
            og_of[og:og + w] = og
            w_of[og:og + w] = w
        ERV = np.zeros((P, NBLK * CPB2 * HID), np.float16)
        rows = reluE[rev[e2]].reshape(nm, 2, P)
        jm, bm, im = j2[m2], blk2[m2], p2[m2]
        cbase = (bm * CPB2 * HID + og_of[jm] * HID + (jm - og_of[jm]) * P + im)
        for r in range(2):
            ERV[:, cbase + r * w_of[jm] * P] = rows[:, r, :].T
        src_row = np.full((1, NBLK * CPB2 * P), PAD_LANE, np.float16)
        src_row[0, col2[m2] * P + p2[m2]] = lane2[m2].astype(np.float16)

        in_maps.append({
            "E_p1": E_p1.reshape(P, NBLK * CPB1 * HID),
            "dest_lanes": dest_lanes,
            "ERV": ERV,
            "src_row": src_row,
            "Wt": Wt_stack,
            "iota_row": iota_row,
            "iota_col": iota_col,
        })
        metas.append({"e2": e2, "p2": p2[m2], "col2": col2[m2]})

    meta = {"CPB1": CPB1, "CPB2": CPB2, "metas": metas, "b": b}
    return in_maps, meta


def build_program(CPB1, CPB2, reps=1):
    f32 = mybir.dt.float32
    f16 = mybir.dt.float16
    nc = bacc.Bacc("TRN2", target_bir_lowering=False, debug=False,
                   num_devices=NC)
    E_p1 = nc.dram_tensor("E_p1", [P, NBLK * CPB1 * HID], f16,
                          kind="ExternalInput").ap()
    dest_lanes = nc.dram_tensor("dest_lanes", [P, NBLK * CPB1], f16,
                                kind="ExternalInput").ap()
    ERV = nc.dram_tensor("ERV", [P, NBLK * CPB2 * HID], f16,
                         kind="ExternalInput").ap()
    src_row = nc.dram_tensor("src_row", [1, NBLK * CPB2 * P], f16,
                             kind="ExternalInput").ap()
    Wt = nc.dram_tensor("Wt", [2, P, HID], f16, kind="ExternalInput").ap()
    iota_row = nc.dram_tensor("iota_row", [P, P], f16,
                              kind="ExternalInput").ap()
    iota_col = nc.dram_tensor("iota_col", [P, 1], f16,
                              kind="ExternalInput").ap()
    out = nc.dram_tensor("out", [P, NBLK * CPB2 * HID], f16,
                         kind="ExternalOutput").ap()

    with tile.TileContext(nc) as tc:
        with ExitStack() as ctx:
            const = ctx.enter_context(tc.tile_pool(name="const", bufs=1))
            sb = ctx.enter_context(tc.tile_pool(name="sb", bufs=SB_BUFS))
            mvp = ctx.enter_context(tc.tile_pool(name="mv", bufs=1))
            ps_mv = ctx.enter_context(
                tc.tile_pool(name="ps_mv", bufs=PS_BUFS[0], space="PSUM"))
            ps_pv = ctx.enter_context(
                tc.tile_pool(name="ps_pv", bufs=PS_BUFS[1], space="PSUM"))
            ps_out = ctx.enter_context(
                tc.tile_pool(name="ps_out", bufs=PS_BUFS[2], space="PSUM"))

            wt0 = const.tile([P, HID], f16)
            nc.sync.dma_start(out=wt0[:], in_=Wt[0])
            wt1 = const.tile([P, HID], f16)
            nc.sync.dma_start(out=wt1[:], in_=Wt[1])
            iota_r = const.tile([P, P], f16)
            nc.sync.dma_start(out=iota_r[:], in_=iota_row[:])
            iota_c = const.tile([P, 1], f16)
            nc.sync.dma_start(out=iota_c[:], in_=iota_col[:])
            dest_t = const.tile([P, NBLK * CPB1], f16)
            nc.sync.dma_start(out=dest_t[:], in_=dest_lanes[:])
            src_sb = const.tile([1, NBLK * CPB2 * P], f16)
            nc.sync.dma_start(out=src_sb[:], in_=src_row[:])

            mv_all = mvp.tile([P, NBLK * HID], f16)  # resident M_v

            env = {
                "sb": sb, "mv_all": mv_all, "ps_mv": ps_mv, "ps_pv": ps_pv,
                "ps_out": ps_out, "E_p1": E_p1, "ERV": ERV, "out": out,
                "dest_t": dest_t, "src_sb": src_sb, "iota_r": iota_r,
                "iota_c": iota_c, "wt0": wt0, "wt1": wt1,
            }
            for _rep in range(reps):
                _emit_body(nc, env, CPB1, CPB2)
    nc.compile()
    return nc


def _emit_p1(nc, env, CPB1, bb):
    f32 = mybir.dt.float32
    f16 = mybir.dt.float16
    sb = env["sb"]
    h_blk = sb.tile([P, CPB1 * HID], f16, tag="h_blk")
    nc.sync.dma_start(
        out=h_blk[:],
        in_=env["E_p1"][:, bb * CPB1 * HID:(bb + 1) * CPB1 * HID])
    mv_ps = env["ps_mv"].tile([P, HID], f32, space="PSUM")
    for j in range(CPB1):
        s_t = sb.tile([P, P], f16, tag="s_t")
        col = bb * CPB1 + j
        nc.vector.tensor_tensor(
            out=s_t[:],
            in0=env["dest_t"][:, col:col + 1].to_broadcast([P, P]),
            in1=env["iota_r"][:],
            op=mybir.AluOpType.is_equal)
        nc.tensor.matmul(
            out=mv_ps[:], lhsT=s_t[:],
            rhs=h_blk[:, j * HID:(j + 1) * HID],
            start=(j == 0), stop=(j == CPB1 - 1))
    nc.scalar.activation(env["mv_all"][:, bb * HID:(bb + 1) * HID], mv_ps[:],
                         mybir.ActivationFunctionType.Copy)


def _emit_p2(nc, env, CPB2, bb):
    f32 = mybir.dt.float32
    f16 = mybir.dt.float16
    sb, mv_all = env["sb"], env["mv_all"]
    erv = sb.tile([P, CPB2 * HID], f16, tag="erv")
    nc.scalar.dma_start(
        out=erv[:], in_=env["ERV"][:, bb * CPB2 * HID:(bb + 1) * CPB2 * HID])
    rb_bc = sb.tile([P, CPB2 * P], f16, tag="rb_bc")
    nc.gpsimd.partition_broadcast(
        rb_bc[:], env["src_sb"][0:1, bb * CPB2 * P:(bb + 1) * CPB2 * P])
    rb = sb.tile([P, CPB2 * P], f16, tag="rb")
    nc.vector.tensor_tensor(
        out=rb[:], in0=rb_bc[:],
        in1=env["iota_c"][:, 0:1].to_broadcast([P, CPB2 * P]),
        op=mybir.AluOpType.is_equal)
    out_blk = sb.tile([P, CPB2 * HID], f16, tag="out_blk")
    for og, w in _groups(CPB2):
        # paired pv: one matmul per Mv half covers w chunks
        pv_ps = env["ps_pv"].tile([P, w * HID], f32, space="PSUM")
        nc.tensor.matmul(
            out=pv_ps[:, 0:w * P], lhsT=mv_all[:, bb * HID:bb * HID + P],
            rhs=rb[:, og * P:(og + w) * P], start=True, stop=True)
        nc.tensor.matmul(
            out=pv_ps[:, w * P:2 * w * P],
            lhsT=mv_all[:, bb * HID + P:(bb + 1) * HID],
            rhs=rb[:, og * P:(og + w) * P], start=True, stop=True)
        muv = sb.tile([P, w * HID], f16, tag="muv")
        nc.vector.tensor_tensor(
            out=muv[:], in0=pv_ps[:],
            in1=erv[:, og * HID:(og + w) * HID],
            op=mybir.AluOpType.subtract)
        out_ps = env["ps_out"].tile([P, w * HID], f32, space="PSUM")
        for jj in range(w):
            nc.tensor.matmul(
                out=out_ps[:, jj * HID:(jj + 1) * HID],
                lhsT=muv[:, jj * P:(jj + 1) * P],
                rhs=env["wt0"][:], start=True, stop=False)
            nc.tensor.matmul(
                out=out_ps[:, jj * HID:(jj + 1) * HID],
                lhsT=muv[:, w * P + jj * P:w * P + (jj + 1) * P],
                rhs=env["wt1"][:], start=False, stop=True)
        nc.scalar.activation(out_blk[:, (og) * HID:(og + w) * HID], out_ps[:],
                             mybir.ActivationFunctionType.Copy)
    # alternate the output-write queue to balance the two HWDGE rings
    eng = nc.sync if bb % 2 == 0 else nc.scalar
    eng.dma_start(
        out=env["out"][:, bb * CPB2 * HID:(bb + 1) * CPB2 * HID],
        in_=out_blk[:])


def _emit_body(nc, env, CPB1, CPB2):
    # software pipeline: phase1(bb+1) is emitted before phase2(bb) so the PE
    # has phase-1 matmuls to chew on while phase2(bb) waits for the Mv copy.
    _emit_p1(nc, env, CPB1, 0)
    for bb in range(NBLK - 1):
        _emit_p1(nc, env, CPB1, bb + 1)
        _emit_p2(nc, env, CPB2, bb)
    _emit_p2(nc, env, CPB2, NBLK - 1)


def assemble(results, meta):
    CPB2 = meta["CPB2"]
    b = meta["b"]
    out_full = np.empty((N_EDGES, HID), np.float32)
    for c in range(NC):
        mc = meta["metas"][c]
        arr = results[c]["out"].reshape(P, NBLK * CPB2, HID)
        out_full[mc["e2"]] = arr[mc["p2"], mc["col2"]]
    out_full += b
    return out_full


def kernel(E, edge_index, rev_index, W, b):
    in_maps, meta = prepare(E, edge_index, rev_index, W, b)
    nc = build_program(meta["CPB1"], meta["CPB2"])
    res = run_bass_kernel_spmd(nc, in_maps, list(range(NC)))
    return assemble(res.results, meta)
